# revision 9
# baseline (speedup 1.0000x reference)
"""Bass/Trainium2 kernel for nn_Graph_Layer (gnn_message_passing).

Reference math (N=8192, D=512):
    G0[i,j] = ||s_i - s_j + eps||_2   (pairwise distances, Gram trick)
    G = 1 - G0 / rowmax(G0)
    out = (G @ x) @ W

Decomposition (row-shard over 8 cores, 1024 rows each). Key identity:
(G @ x) @ W = G @ (x @ W), so the weight GEMM folds into a host-side
precompute xw = x @ W and the device only does:
    sqd[i,j] = ri[i] + cj[j] - 2*gram[i,j]     (ri, cj host-precomputed)
    G0 = sqrt(sqd + CLAMP)                      (CLAMP covers tf32 noise on diag)
    rowmax[i] = max_j G0[i,j]
    out[i,:]  = w2 - (G0 @ xw)[i,:]/rowmax[i],  w2 = colsum_x @ W (host)

On device the distance strip is computed TRANSPOSED (sqd^T[j,i]) so the
G0 tiles come out with j (the contraction dim of Y = G0 @ xw) on
partitions -- no transposes of G0 needed. cj[j] rides the ACT sqrt bias
(per-partition); ri[i] varies along the free dim so it is added by DVE
from a host-replicated [128, R] tile (avoids a 512-cycle aug matmul per
j-tile). Each core sees its own np.roll'ed copy of the inputs so the
"local rows" are always rows [0,1024): one uniform SPMD program.

All matmuls use float32r (TF32 mode: 1 cycle/row at free dim >= 512).
"""

import numpy as np
from contextlib import ExitStack

import concourse.bass as bass
from concourse import bacc
import concourse.tile as tile
from concourse import mybir
from concourse.bass_utils import run_bass_kernel_spmd
from concourse.masks import make_identity

N, D, NOUT = 8192, 512, 512
M = 8                 # cores
R = N // M            # 1024 local rows per core
EPS = 1e-6
CLAMP = 0.3           # covers tf32 rounding noise on the diagonal; ~1e-4 rel effect off-diag
F32 = mybir.dt.float32
F32R = mybir.dt.float32r

KT = D // 128         # 4 contraction sub-tiles
NJT = N // 128        # 64 j tiles
IB = 512              # i block (free dim of the gram matmuls)
NIB = R // IB         # 2
NSUB = IB // 128      # 4 sub-tiles of 128 rows per i block

CH = 512              # S^T DMA chunk width (columns); chunk c covers j_tiles 4c..4c+3
NCH = N // CH


def build_kernel(ctx, tc, out_d, xw_d, s_d, cj_d, ri_d, w2_d):
    nc = tc.nc

    singles = ctx.enter_context(tc.tile_pool(name="singles", bufs=1))
    xt_pool = ctx.enter_context(tc.tile_pool(name="xt", bufs=4))
    g0_pool = ctx.enter_context(tc.tile_pool(name="g0", bufs=4))
    t_pool = ctx.enter_context(tc.tile_pool(name="t", bufs=3))
    osb_pool = ctx.enter_context(tc.tile_pool(name="osb", bufs=4))
    sm_pool = ctx.enter_context(tc.tile_pool(name="sm", bufs=4))
    macc_pool = ctx.enter_context(tc.tile_pool(name="macc", bufs=2))
    ps_tr = ctx.enter_context(tc.tile_pool(name="ps_tr", bufs=1, space="PSUM"))
    ps_g = ctx.enter_context(tc.tile_pool(name="ps_g", bufs=3, space="PSUM"))
    ps_y = ctx.enter_context(tc.tile_pool(name="ps_y", bufs=1, space="PSUM"))

    # --- persistent SBUF tensors ---
    st = singles.tile([128, KT * N], F32R)            # S^T: [k*N + j] layout
    cj_sb = singles.tile([128, NJT], F32)             # cj[t*128+p] at [p, t]
    ri_sb = singles.tile([128, R], F32)               # -ri/2, replicated rows
    w2_sb = singles.tile([128, NOUT], F32)            # w2 replicated rows
    ident = singles.tile([128, 128], F32)

    make_identity(nc, ident[:])

    def load_st_chunk(c):
        for k in range(KT):
            nc.sync.dma_start(
                out=st[:, k * N + c * CH: k * N + (c + 1) * CH],
                in_=s_d[bass.ts(k, 128), c * CH:(c + 1) * CH].bitcast(F32R),
            )

    load_st_chunk(0)
    nc.sync.dma_start(out=cj_sb[:], in_=cj_d)

    # --- main: per i-block: gram strip -> G0 -> Y accum -> normalize ---
    for ib in range(NIB):
        icol0 = ib * IB  # local column offset into S^T / ri
        psy = [ps_y.tile([128, NOUT], F32, tag=f"y{s}", name=f"psy{s}")
               for s in range(NSUB)]
        macc = macc_pool.tile([128, IB], F32, tag="macc")
        pipe = []

        for jt in range(NJT):
            xt = xt_pool.tile([128, NOUT], F32R, tag="xt")
            nc.sync.dma_start(out=xt[:], in_=xw_d[bass.ts(jt, 128), :].bitcast(F32R))

            if ib == 0:
                if jt == 0:
                    nc.sync.dma_start(out=ri_sb[:], in_=ri_d)
                    load_st_chunk(1)
                    load_st_chunk(2)
                elif jt % 4 == 0 and jt // 4 + 2 < NCH:
                    load_st_chunk(jt // 4 + 2)
                if jt == 32:
                    nc.sync.dma_start(out=w2_sb[:], in_=w2_d)

            psg = ps_g.tile([128, IB], F32, tag="g")
            for k in range(KT):
                nc.tensor.matmul(
                    psg[:],
                    st[:, k * N + jt * 128: k * N + jt * 128 + 128],
                    st[:, k * N + icol0: k * N + icol0 + IB],
                    start=(k == 0),
                    stop=(k == KT - 1),
                )

            # t = psg + (-ri/2)  (free-dim-varying term, DVE broadcast-free add)
            t = t_pool.tile([128, IB], F32, tag="t")
            nc.vector.tensor_add(t[:], psg[:], ri_sb[:, icol0:icol0 + IB])

            # G0^T tile = sqrt(-2*t + cj[j])   (cj includes +CLAMP)
            g0 = g0_pool.tile([128, IB], F32R, tag="g0")
            nc.scalar.activation(
                out=g0[:], in_=t[:],
                func=mybir.ActivationFunctionType.Sqrt,
                bias=cj_sb[:, jt:jt + 1], scale=-2.0,
            )

            if jt == 0:
                nc.vector.tensor_copy(out=macc[:], in_=g0[:].bitcast(F32))
            else:
                nc.vector.tensor_max(macc[:], macc[:], g0[:].bitcast(F32))

            # software pipeline: issue Y matmuls two steps behind the gram so
            # the PE has a full j-tile of slack over the DVE/ACT latency
            if jt > 1:
                pg0, pxt = pipe.pop(0)
                for s in range(NSUB):
                    nc.tensor.matmul(
                        psy[s][:], pg0[:, bass.ts(s, 128)], pxt[:],
                        start=(jt == 2), stop=False,
                    )
            pipe.append((g0, xt))

        for tail_i, (pg0, pxt) in enumerate(pipe):
            for s in range(NSUB):
                nc.tensor.matmul(
                    psy[s][:], pg0[:, bass.ts(s, 128)], pxt[:],
                    start=False, stop=(tail_i == len(pipe) - 1),
                )

        # tail: rowmax -> -1/rowmax -> ysc = -Y/rowmax -> out = w2 + ysc
        for s in range(NSUB):
            pst = ps_tr.tile([128, 128], F32, tag="tr")
            nc.tensor.transpose(pst[:], macc[:, bass.ts(s, 128)], ident[:])
            rm = sm_pool.tile([128, 1], F32, tag="rm")
            nc.vector.tensor_reduce(
                out=rm[:], in_=pst[:], axis=mybir.AxisListType.X,
                op=mybir.AluOpType.max,
            )
            nrm = sm_pool.tile([128, 1], F32, tag="nrm")
            nc.vector.tensor_scalar_mul(nrm[:], rm[:], -1.0)
            ninv = sm_pool.tile([128, 1], F32, tag="ninv")
            nc.vector.reciprocal(ninv[:], nrm[:])  # -1/rowmax

            # out = (psy * (-1/rowmax)) + w2, one DVE op draining PSUM
            osb = osb_pool.tile([128, NOUT], F32, tag="osb")
            nc.vector.scalar_tensor_tensor(
                out=osb[:], in0=psy[s][:], scalar=ninv[:], in1=w2_sb[:],
                op0=mybir.AluOpType.mult, op1=mybir.AluOpType.add,
            )
            nc.sync.dma_start(out=out_d[bass.ts(ib * NSUB + s, 128), :], in_=osb[:])


_NC_CACHE = {}


def _build_nc():
    if "nc" in _NC_CACHE:
        return _NC_CACHE["nc"]
    nc = bacc.Bacc("TRN2", target_bir_lowering=False, debug=False, num_devices=M)
    xw_d = nc.dram_tensor("xw", [N, NOUT], F32, kind="ExternalInput").ap()
    s_d = nc.dram_tensor("simT", [D, N], F32, kind="ExternalInput").ap()
    cj_d = nc.dram_tensor("cj", [128, NJT], F32, kind="ExternalInput").ap()
    ri_d = nc.dram_tensor("rirep", [128, R], F32, kind="ExternalInput").ap()
    w2_d = nc.dram_tensor("w2rep", [128, NOUT], F32, kind="ExternalInput").ap()
    out_d = nc.dram_tensor("out", [R, NOUT], F32, kind="ExternalOutput").ap()
    with tile.TileContext(nc) as tc, ExitStack() as ctx:
        build_kernel(ctx, tc, out_d, xw_d, s_d, cj_d, ri_d, w2_d)
    nc.compile()
    _NC_CACHE["nc"] = nc
    return nc


def make_in_maps(x, sim_feat, weight):
    x = np.ascontiguousarray(x, dtype=np.float32)
    sim = np.ascontiguousarray(sim_feat, dtype=np.float32)
    w = np.ascontiguousarray(weight, dtype=np.float32)

    sim64 = sim.astype(np.float64)
    sq = (sim64 * sim64).sum(1)
    ss = sim64.sum(1)
    cj_full = (sq - 2.0 * EPS * ss + CLAMP).astype(np.float32)         # [N]
    ri_full = sq + 2.0 * EPS * ss + D * EPS * EPS                      # [N] f64
    colsum = x.astype(np.float64).sum(0)
    w2 = (colsum @ w.astype(np.float64)).astype(np.float32)
    xw = np.ascontiguousarray(x @ w)                                   # [N, NOUT] f32 sgemm
    w2_rep = np.ascontiguousarray(np.broadcast_to(w2, (128, NOUT)))

    in_maps = []
    for c in range(M):
        shift = c * R
        sim_c = np.ascontiguousarray(np.roll(sim, -shift, axis=0).T)
        xw_c = np.roll(xw, -shift, axis=0)
        cj_c = np.ascontiguousarray(
            np.roll(cj_full, -shift).reshape(NJT, 128).T
        )                                                               # [128, NJT]
        ri_c = np.ascontiguousarray(np.broadcast_to(
            (-(ri_full[shift:shift + R]) / 2.0).astype(np.float32), (128, R)
        ))
        in_maps.append(
            {"xw": xw_c, "simT": sim_c, "cj": cj_c, "rirep": ri_c,
             "w2rep": w2_rep}
        )
    return in_maps


def kernel(x, sim_feat, weight, _trace=False, **kw):
    nc = _build_nc()
    in_maps = make_in_maps(x, sim_feat, weight)
    res = run_bass_kernel_spmd(nc, in_maps, list(range(M)), trace=_trace, **kw)
    out = np.concatenate([res.results[c]["out"] for c in range(M)], axis=0)
    if _trace:
        return out, res
    return out


# revision 12
# speedup vs baseline: 1.0009x; 1.0009x over previous
"""Bass/Trainium2 kernel for nn_Graph_Layer (gnn_message_passing).

Reference math (N=8192, D=512):
    G0[i,j] = ||s_i - s_j + eps||_2   (pairwise distances, Gram trick)
    G = 1 - G0 / rowmax(G0)
    out = (G @ x) @ W

Decomposition (row-shard over 8 cores, 1024 rows each). Key identity:
(G @ x) @ W = G @ (x @ W), so the weight GEMM folds into a host-side
precompute xw = x @ W and the device only does:
    sqd[i,j] = ri[i] + cj[j] - 2*gram[i,j]     (ri, cj host-precomputed)
    G0 = sqrt(sqd + CLAMP)                      (CLAMP covers tf32 noise on diag)
    rowmax[i] = max_j G0[i,j]
    out[i,:]  = w2 - (G0 @ xw)[i,:]/rowmax[i],  w2 = colsum_x @ W (host)

On device the distance strip is computed TRANSPOSED (sqd^T[j,i]) so the
G0 tiles come out with j (the contraction dim of Y = G0 @ xw) on
partitions -- no transposes of G0 needed. cj[j] rides the ACT sqrt bias
(per-partition); ri[i] varies along the free dim so it is added by DVE
from a host-replicated [128, R] tile (avoids a 512-cycle aug matmul per
j-tile). Each core sees its own np.roll'ed copy of the inputs so the
"local rows" are always rows [0,1024): one uniform SPMD program.

All matmuls use float32r (TF32 mode: 1 cycle/row at free dim >= 512).
"""

import numpy as np
from contextlib import ExitStack

import concourse.bass as bass
from concourse import bacc
import concourse.tile as tile
from concourse import mybir
from concourse.bass_utils import run_bass_kernel_spmd
from concourse.masks import make_identity

N, D, NOUT = 8192, 512, 512
M = 8                 # cores
R = N // M            # 1024 local rows per core
EPS = 1e-6
CLAMP = 0.3           # covers tf32 rounding noise on the diagonal; ~1e-4 rel effect off-diag
F32 = mybir.dt.float32
F32R = mybir.dt.float32r

KT = D // 128         # 4 contraction sub-tiles
NJT = N // 128        # 64 j tiles
IB = 512              # i block (free dim of the gram matmuls)
NIB = R // IB         # 2
NSUB = IB // 128      # 4 sub-tiles of 128 rows per i block

CH = 512              # S^T DMA chunk width (columns); chunk c covers j_tiles 4c..4c+3
NCH = N // CH


def build_kernel(ctx, tc, out_d, xw_d, s_d, cj_d, ri_d, w2_d):
    nc = tc.nc

    singles = ctx.enter_context(tc.tile_pool(name="singles", bufs=1))
    xt_pool = ctx.enter_context(tc.tile_pool(name="xt", bufs=4))
    g0_pool = ctx.enter_context(tc.tile_pool(name="g0", bufs=4))
    t_pool = ctx.enter_context(tc.tile_pool(name="t", bufs=3))
    osb_pool = ctx.enter_context(tc.tile_pool(name="osb", bufs=4))
    sm_pool = ctx.enter_context(tc.tile_pool(name="sm", bufs=4))
    macc_pool = ctx.enter_context(tc.tile_pool(name="macc", bufs=2))
    ps_tr = ctx.enter_context(tc.tile_pool(name="ps_tr", bufs=1, space="PSUM"))
    ps_g = ctx.enter_context(tc.tile_pool(name="ps_g", bufs=3, space="PSUM"))
    ps_y = ctx.enter_context(tc.tile_pool(name="ps_y", bufs=1, space="PSUM"))

    # --- persistent SBUF tensors ---
    st = singles.tile([128, KT * N], F32R)            # S^T: [k*N + j] layout
    cj_sb = singles.tile([128, NJT], F32)             # cj[t*128+p] at [p, t]
    ri_sb = singles.tile([128, R], F32)               # -ri/2, replicated rows
    w2_sb = singles.tile([128, NOUT], F32)            # w2 replicated rows
    ident = singles.tile([128, 128], F32)

    make_identity(nc, ident[:])

    def load_st_chunk(c):
        for k in range(KT):
            nc.sync.dma_start(
                out=st[:, k * N + c * CH: k * N + (c + 1) * CH],
                in_=s_d[bass.ts(k, 128), c * CH:(c + 1) * CH].bitcast(F32R),
            )

    load_st_chunk(0)
    nc.sync.dma_start(out=cj_sb[:], in_=cj_d)

    # --- main: per i-block: gram strip -> G0 -> Y accum -> normalize ---
    for ib in range(NIB):
        icol0 = ib * IB  # local column offset into S^T / ri
        psy = [ps_y.tile([128, NOUT], F32, tag=f"y{s}", name=f"psy{s}")
               for s in range(NSUB)]
        macc = macc_pool.tile([128, IB], F32, tag="macc")
        pipe = []

        for jt in range(NJT):
            xt = xt_pool.tile([128, NOUT], F32R, tag="xt")
            nc.sync.dma_start(out=xt[:], in_=xw_d[bass.ts(jt, 128), :].bitcast(F32R))

            if ib == 0:
                if jt == 0:
                    nc.sync.dma_start(out=ri_sb[:], in_=ri_d)
                    load_st_chunk(1)
                else:
                    # one k-slice per j-tile; chunk c complete at jt=4c-4,
                    # consumed from jt=4c (one-chunk slack, smooth DMA rate)
                    q = jt - 1
                    c = 2 + q // 4
                    if c < NCH:
                        k = q % 4
                        nc.sync.dma_start(
                            out=st[:, k * N + c * CH: k * N + (c + 1) * CH],
                            in_=s_d[bass.ts(k, 128), c * CH:(c + 1) * CH].bitcast(F32R),
                        )
                if jt == 32:
                    nc.sync.dma_start(out=w2_sb[:], in_=w2_d)

            psg = ps_g.tile([128, IB], F32, tag="g")
            for k in range(KT):
                nc.tensor.matmul(
                    psg[:],
                    st[:, k * N + jt * 128: k * N + jt * 128 + 128],
                    st[:, k * N + icol0: k * N + icol0 + IB],
                    start=(k == 0),
                    stop=(k == KT - 1),
                )

            # t = psg + (-ri/2)  (free-dim-varying term, DVE broadcast-free add)
            t = t_pool.tile([128, IB], F32, tag="t")
            nc.vector.tensor_add(t[:], psg[:], ri_sb[:, icol0:icol0 + IB])

            # G0^T tile = sqrt(-2*t + cj[j])   (cj includes +CLAMP)
            g0 = g0_pool.tile([128, IB], F32R, tag="g0")
            nc.scalar.activation(
                out=g0[:], in_=t[:],
                func=mybir.ActivationFunctionType.Sqrt,
                bias=cj_sb[:, jt:jt + 1], scale=-2.0,
            )

            if jt == 0:
                nc.vector.tensor_copy(out=macc[:], in_=g0[:].bitcast(F32))
            else:
                nc.vector.tensor_max(macc[:], macc[:], g0[:].bitcast(F32))

            # software pipeline: issue Y matmuls two steps behind the gram so
            # the PE has a full j-tile of slack over the DVE/ACT latency
            if jt > 1:
                pg0, pxt = pipe.pop(0)
                for s in range(NSUB):
                    nc.tensor.matmul(
                        psy[s][:], pg0[:, bass.ts(s, 128)], pxt[:],
                        start=(jt == 2), stop=False,
                    )
            pipe.append((g0, xt))

        for tail_i, (pg0, pxt) in enumerate(pipe):
            for s in range(NSUB):
                nc.tensor.matmul(
                    psy[s][:], pg0[:, bass.ts(s, 128)], pxt[:],
                    start=False, stop=(tail_i == len(pipe) - 1),
                )

        # tail: rowmax -> -1/rowmax -> out = w2 + psy * (-1/rowmax).
        # All 4 transposes land in one PSUM bank (disjoint 128-col ranges)
        # and all reduces are issued before the big stt ops so the rowmax
        # chain isn't FIFO-blocked behind them on DVE.
        pst = ps_tr.tile([128, IB], F32, tag="tr")
        for s in range(NSUB):
            nc.tensor.transpose(
                pst[:, bass.ts(s, 128)], macc[:, bass.ts(s, 128)], ident[:])
        ninvs = []
        for s in range(NSUB):
            rm = sm_pool.tile([128, 1], F32, tag="rm")
            nc.vector.tensor_reduce(
                out=rm[:], in_=pst[:, bass.ts(s, 128)], axis=mybir.AxisListType.X,
                op=mybir.AluOpType.max,
            )
            nrm = sm_pool.tile([128, 1], F32, tag="nrm")
            nc.vector.tensor_scalar_mul(nrm[:], rm[:], -1.0)
            ninv = sm_pool.tile([128, 1], F32, tag="ninv")
            nc.vector.reciprocal(ninv[:], nrm[:])  # -1/rowmax
            ninvs.append(ninv)
        for s in range(NSUB):
            osb = osb_pool.tile([128, NOUT], F32, tag="osb")
            nc.vector.scalar_tensor_tensor(
                out=osb[:], in0=psy[s][:], scalar=ninvs[s][:], in1=w2_sb[:],
                op0=mybir.AluOpType.mult, op1=mybir.AluOpType.add,
            )
            nc.sync.dma_start(out=out_d[bass.ts(ib * NSUB + s, 128), :], in_=osb[:])


_NC_CACHE = {}


def _build_nc():
    if "nc" in _NC_CACHE:
        return _NC_CACHE["nc"]
    nc = bacc.Bacc("TRN2", target_bir_lowering=False, debug=False, num_devices=M)
    xw_d = nc.dram_tensor("xw", [N, NOUT], F32, kind="ExternalInput").ap()
    s_d = nc.dram_tensor("simT", [D, N], F32, kind="ExternalInput").ap()
    cj_d = nc.dram_tensor("cj", [128, NJT], F32, kind="ExternalInput").ap()
    ri_d = nc.dram_tensor("rirep", [128, R], F32, kind="ExternalInput").ap()
    w2_d = nc.dram_tensor("w2rep", [128, NOUT], F32, kind="ExternalInput").ap()
    out_d = nc.dram_tensor("out", [R, NOUT], F32, kind="ExternalOutput").ap()
    with tile.TileContext(nc) as tc, ExitStack() as ctx:
        build_kernel(ctx, tc, out_d, xw_d, s_d, cj_d, ri_d, w2_d)
    nc.compile()
    _NC_CACHE["nc"] = nc
    return nc


def make_in_maps(x, sim_feat, weight):
    x = np.ascontiguousarray(x, dtype=np.float32)
    sim = np.ascontiguousarray(sim_feat, dtype=np.float32)
    w = np.ascontiguousarray(weight, dtype=np.float32)

    sim64 = sim.astype(np.float64)
    sq = (sim64 * sim64).sum(1)
    ss = sim64.sum(1)
    cj_full = (sq - 2.0 * EPS * ss + CLAMP).astype(np.float32)         # [N]
    ri_full = sq + 2.0 * EPS * ss + D * EPS * EPS                      # [N] f64
    colsum = x.astype(np.float64).sum(0)
    w2 = (colsum @ w.astype(np.float64)).astype(np.float32)
    xw = np.ascontiguousarray(x @ w)                                   # [N, NOUT] f32 sgemm
    w2_rep = np.ascontiguousarray(np.broadcast_to(w2, (128, NOUT)))

    in_maps = []
    for c in range(M):
        shift = c * R
        sim_c = np.ascontiguousarray(np.roll(sim, -shift, axis=0).T)
        xw_c = np.roll(xw, -shift, axis=0)
        cj_c = np.ascontiguousarray(
            np.roll(cj_full, -shift).reshape(NJT, 128).T
        )                                                               # [128, NJT]
        ri_c = np.ascontiguousarray(np.broadcast_to(
            (-(ri_full[shift:shift + R]) / 2.0).astype(np.float32), (128, R)
        ))
        in_maps.append(
            {"xw": xw_c, "simT": sim_c, "cj": cj_c, "rirep": ri_c,
             "w2rep": w2_rep}
        )
    return in_maps


def kernel(x, sim_feat, weight, _trace=False, **kw):
    nc = _build_nc()
    in_maps = make_in_maps(x, sim_feat, weight)
    res = run_bass_kernel_spmd(nc, in_maps, list(range(M)), trace=_trace, **kw)
    out = np.concatenate([res.results[c]["out"] for c in range(M)], axis=0)
    if _trace:
        return out, res
    return out


# revision 15
# speedup vs baseline: 1.0925x; 1.0915x over previous
"""Bass/Trainium2 kernel for nn_Graph_Layer (gnn_message_passing).

Reference math (N=8192, D=512):
    G0[i,j] = ||s_i - s_j + eps||_2   (pairwise distances, Gram trick)
    G = 1 - G0 / rowmax(G0)
    out = (G @ x) @ W

Decomposition (row-shard over 8 cores, 1024 rows each). Key identity:
(G @ x) @ W = G @ (x @ W), so the weight GEMM folds into a host-side
precompute xw = x @ W and the device only does:
    sqd[i,j] = ri[i] + cj[j] - 2*gram[i,j]     (ri, cj host-precomputed)
    G0 = sqrt(sqd + CLAMP)                      (CLAMP covers fp16 noise on diag)
    rowmax[i] = max_j G0[i,j]
    out[i,:]  = w2 - (G0 @ xw)[i,:]/rowmax[i],  w2 = colsum_x @ W (host)

On device the distance strip is computed TRANSPOSED (sqd^T[j,i]) so the
G0 tiles come out with j (the contraction dim of Y = G0 @ xw) on
partitions -- no transposes of G0 needed. cj[j] rides the ACT sqrt bias
(per-partition); ri[i] varies along the free dim so it is added by DVE
from a host-replicated [128, R] tile (avoids a 512-cycle aug matmul per
j-tile).

All matmuls run in fp16 (1 cycle/row + fast weight load; verified
rel err ~5e-3 vs the 2e-2 gate). s^T and xw live in SBUF in fp16, so
HBM traffic is ~16 MB/core. Each core sees its own np.roll'ed copy of
the inputs so the "local rows" are always rows [0,1024): one uniform
SPMD program runs on all 8 cores.
"""

import numpy as np
from contextlib import ExitStack

import concourse.bass as bass
from concourse import bacc
import concourse.tile as tile
from concourse import mybir
from concourse.bass_utils import run_bass_kernel_spmd
from concourse.masks import make_identity

N, D, NOUT = 8192, 512, 512
M = 8                 # cores
R = N // M            # 1024 local rows per core
EPS = 1e-6
CLAMP = 0.3           # keeps the sqrt arg positive under fp16 gram noise
F32 = mybir.dt.float32
F16 = mybir.dt.float16

KT = D // 128         # 4 contraction sub-tiles
NJT = N // 128        # 64 j tiles
IB = 512              # i block (free dim of the gram matmuls)
NIB = R // IB         # 2
NSUB = IB // 128      # 4 sub-tiles of 128 rows per i block

CH = 512              # S^T DMA chunk width (columns); chunk c covers j_tiles 4c..4c+3
NCH = N // CH


def build_kernel(ctx, tc, out_d, xw_d, s_d, cj_d, ri_d, w2_d):
    nc = tc.nc

    singles = ctx.enter_context(tc.tile_pool(name="singles", bufs=1))
    g0_pool = ctx.enter_context(tc.tile_pool(name="g0", bufs=4))
    t_pool = ctx.enter_context(tc.tile_pool(name="t", bufs=3))
    osb_pool = ctx.enter_context(tc.tile_pool(name="osb", bufs=4))
    sm_pool = ctx.enter_context(tc.tile_pool(name="sm", bufs=4))
    macc_pool = ctx.enter_context(tc.tile_pool(name="macc", bufs=2))
    ps_tr = ctx.enter_context(tc.tile_pool(name="ps_tr", bufs=1, space="PSUM"))
    ps_g = ctx.enter_context(tc.tile_pool(name="ps_g", bufs=3, space="PSUM"))
    ps_y = ctx.enter_context(tc.tile_pool(name="ps_y", bufs=1, space="PSUM"))

    # --- persistent SBUF tensors ---
    st = singles.tile([128, KT * N], F16)             # S^T: [k*N + j] layout
    xw_sb = singles.tile([128, NJT * NOUT], F16)      # all xw tiles, resident
    cj_sb = singles.tile([128, NJT], F32)             # cj[t*128+p] at [p, t]
    ri_sb = singles.tile([128, R], F32)               # -ri/2, replicated rows
    w2_sb = singles.tile([128, NOUT], F32)            # w2 replicated rows
    ident = singles.tile([128, 128], F16)

    make_identity(nc, ident[:])

    def load_st_chunk(c):
        for k in range(KT):
            nc.sync.dma_start(
                out=st[:, k * N + c * CH: k * N + (c + 1) * CH],
                in_=s_d[bass.ts(k, 128), c * CH:(c + 1) * CH],
            )

    load_st_chunk(0)
    nc.sync.dma_start(out=cj_sb[:], in_=cj_d)

    # --- main: per i-block: gram strip -> G0 -> Y accum -> normalize ---
    for ib in range(NIB):
        icol0 = ib * IB  # local column offset into S^T / ri
        psy = [ps_y.tile([128, NOUT], F32, tag=f"y{s}", name=f"psy{s}")
               for s in range(NSUB)]
        macc = macc_pool.tile([128, IB], F16, tag="macc")
        pipe = []

        for jt in range(NJT):
            if ib == 0:
                nc.sync.dma_start(
                    out=xw_sb[:, jt * NOUT:(jt + 1) * NOUT],
                    in_=xw_d[bass.ts(jt, 128), :],
                )
                if jt == 0:
                    nc.sync.dma_start(out=ri_sb[:], in_=ri_d)
                    load_st_chunk(1)
                else:
                    # one k-slice per j-tile; chunk c complete at jt=4c-4,
                    # consumed from jt=4c (one-chunk slack, smooth DMA rate)
                    q = jt - 1
                    c = 2 + q // 4
                    if c < NCH:
                        k = q % 4
                        nc.sync.dma_start(
                            out=st[:, k * N + c * CH: k * N + (c + 1) * CH],
                            in_=s_d[bass.ts(k, 128), c * CH:(c + 1) * CH],
                        )
                if jt == 32:
                    nc.sync.dma_start(out=w2_sb[:], in_=w2_d)

            psg = ps_g.tile([128, IB], F32, tag="g")
            for k in range(KT):
                nc.tensor.matmul(
                    psg[:],
                    st[:, k * N + jt * 128: k * N + jt * 128 + 128],
                    st[:, k * N + icol0: k * N + icol0 + IB],
                    start=(k == 0),
                    stop=(k == KT - 1),
                )

            # t = psg + (-ri/2)  (free-dim-varying term, DVE add)
            t = t_pool.tile([128, IB], F32, tag="t")
            nc.vector.tensor_add(t[:], psg[:], ri_sb[:, icol0:icol0 + IB])

            # G0^T tile = sqrt(-2*t + cj[j])   (cj includes +CLAMP)
            g0 = g0_pool.tile([128, IB], F16, tag="g0")
            nc.scalar.activation(
                out=g0[:], in_=t[:],
                func=mybir.ActivationFunctionType.Sqrt,
                bias=cj_sb[:, jt:jt + 1], scale=-2.0,
            )

            if jt == 0:
                nc.vector.tensor_copy(out=macc[:], in_=g0[:])
            else:
                nc.vector.tensor_max(macc[:], macc[:], g0[:])

            # software pipeline: issue Y matmuls two steps behind the gram so
            # the PE has a full j-tile of slack over the DVE/ACT latency
            if jt > 1:
                pg0, pjt = pipe.pop(0)
                for s in range(NSUB):
                    nc.tensor.matmul(
                        psy[s][:], pg0[:, bass.ts(s, 128)],
                        xw_sb[:, pjt * NOUT:(pjt + 1) * NOUT],
                        start=(jt == 2), stop=False,
                    )
            pipe.append((g0, jt))

        for tail_i, (pg0, pjt) in enumerate(pipe):
            for s in range(NSUB):
                nc.tensor.matmul(
                    psy[s][:], pg0[:, bass.ts(s, 128)],
                    xw_sb[:, pjt * NOUT:(pjt + 1) * NOUT],
                    start=False, stop=(tail_i == len(pipe) - 1),
                )

        # tail: rowmax -> -1/rowmax -> out = w2 + psy * (-1/rowmax).
        # All 4 transposes land in one PSUM bank (disjoint 128-col ranges)
        # and all reduces are issued before the big stt ops so the rowmax
        # chain isn't FIFO-blocked behind them on DVE.
        pst = ps_tr.tile([128, IB], F16, tag="tr")
        for s in range(NSUB):
            nc.tensor.transpose(
                pst[:, bass.ts(s, 128)], macc[:, bass.ts(s, 128)], ident[:])
        ninvs = []
        for s in range(NSUB):
            rm = sm_pool.tile([128, 1], F32, tag="rm")
            nc.vector.tensor_reduce(
                out=rm[:], in_=pst[:, bass.ts(s, 128)], axis=mybir.AxisListType.X,
                op=mybir.AluOpType.max,
            )
            nrm = sm_pool.tile([128, 1], F32, tag="nrm")
            nc.vector.tensor_scalar_mul(nrm[:], rm[:], -1.0)
            ninv = sm_pool.tile([128, 1], F32, tag="ninv")
            nc.vector.reciprocal(ninv[:], nrm[:])  # -1/rowmax
            ninvs.append(ninv)
        for s in range(NSUB):
            osb = osb_pool.tile([128, NOUT], F32, tag="osb")
            nc.vector.scalar_tensor_tensor(
                out=osb[:], in0=psy[s][:], scalar=ninvs[s][:], in1=w2_sb[:],
                op0=mybir.AluOpType.mult, op1=mybir.AluOpType.add,
            )
            nc.sync.dma_start(out=out_d[bass.ts(ib * NSUB + s, 128), :], in_=osb[:])


_NC_CACHE = {}


def _build_nc():
    if "nc" in _NC_CACHE:
        return _NC_CACHE["nc"]
    nc = bacc.Bacc("TRN2", target_bir_lowering=False, debug=False, num_devices=M)
    xw_d = nc.dram_tensor("xw", [N, NOUT], F16, kind="ExternalInput").ap()
    s_d = nc.dram_tensor("simT", [D, N], F16, kind="ExternalInput").ap()
    cj_d = nc.dram_tensor("cj", [128, NJT], F32, kind="ExternalInput").ap()
    ri_d = nc.dram_tensor("rirep", [128, R], F32, kind="ExternalInput").ap()
    w2_d = nc.dram_tensor("w2rep", [128, NOUT], F32, kind="ExternalInput").ap()
    out_d = nc.dram_tensor("out", [R, NOUT], F32, kind="ExternalOutput").ap()
    with tile.TileContext(nc) as tc, ExitStack() as ctx:
        build_kernel(ctx, tc, out_d, xw_d, s_d, cj_d, ri_d, w2_d)
    nc.compile()
    _NC_CACHE["nc"] = nc
    return nc


def make_in_maps(x, sim_feat, weight):
    x = np.ascontiguousarray(x, dtype=np.float32)
    sim = np.ascontiguousarray(sim_feat, dtype=np.float32)
    w = np.ascontiguousarray(weight, dtype=np.float32)

    sim64 = sim.astype(np.float64)
    sq = (sim64 * sim64).sum(1)
    ss = sim64.sum(1)
    cj_full = (sq - 2.0 * EPS * ss + CLAMP).astype(np.float32)         # [N]
    ri_full = sq + 2.0 * EPS * ss + D * EPS * EPS                      # [N] f64
    colsum = x.astype(np.float64).sum(0)
    w2 = (colsum @ w.astype(np.float64)).astype(np.float32)
    xw = (x @ w).astype(np.float16)                                    # [N, NOUT]
    sim16 = sim.astype(np.float16)
    w2_rep = np.ascontiguousarray(np.broadcast_to(w2, (128, NOUT)))

    in_maps = []
    for c in range(M):
        shift = c * R
        sim_c = np.ascontiguousarray(np.roll(sim16, -shift, axis=0).T)
        xw_c = np.roll(xw, -shift, axis=0)
        cj_c = np.ascontiguousarray(
            np.roll(cj_full, -shift).reshape(NJT, 128).T
        )                                                               # [128, NJT]
        ri_c = np.ascontiguousarray(np.broadcast_to(
            (-(ri_full[shift:shift + R]) / 2.0).astype(np.float32), (128, R)
        ))
        in_maps.append(
            {"xw": xw_c, "simT": sim_c, "cj": cj_c, "rirep": ri_c,
             "w2rep": w2_rep}
        )
    return in_maps


def kernel(x, sim_feat, weight, _trace=False, **kw):
    nc = _build_nc()
    in_maps = make_in_maps(x, sim_feat, weight)
    res = run_bass_kernel_spmd(nc, in_maps, list(range(M)), trace=_trace, **kw)
    out = np.concatenate([res.results[c]["out"] for c in range(M)], axis=0)
    if _trace:
        return out, res
    return out


# revision 29
# speedup vs baseline: 1.1083x; 1.0145x over previous
"""Bass/Trainium2 kernel for nn_Graph_Layer (gnn_message_passing).

Reference math (N=8192, D=512):
    G0[i,j] = ||s_i - s_j + eps||_2   (pairwise distances, Gram trick)
    G = 1 - G0 / rowmax(G0)
    out = (G @ x) @ W

Decomposition (row-shard over 8 cores, 1024 rows each). Key identity:
(G @ x) @ W = G @ (x @ W), so the weight GEMM folds into a host-side
precompute xw = x @ W and the device only does:
    sqd[i,j] = ri[i] + cj[j] - 2*gram[i,j]     (ri, cj host-precomputed)
    G0 = sqrt(sqd + CLAMP)                      (CLAMP covers fp16 noise on diag)
    rowmax[i] = max_j G0[i,j]
    out[i,:]  = w2 - (G0 @ xw)[i,:]/rowmax[i],  w2 = colsum_x @ W (host)

On device the distance strip is computed TRANSPOSED (sqd^T[j,i]) so the
G0 tiles come out with j (the contraction dim of Y = G0 @ xw) on
partitions -- no transposes of G0 needed. cj[j] rides the ACT sqrt bias
(per-partition); ri[i] varies along the free dim so it is added by DVE
from a host-replicated [128, R] tile (avoids a 512-cycle aug matmul per
j-tile).

All matmuls run in fp16 (1 cycle/row + fast weight load; verified
rel err ~5e-3 vs the 2e-2 gate). s^T and xw live in SBUF in fp16, so
HBM traffic is ~16 MB/core. Each core sees its own np.roll'ed copy of
the inputs so the "local rows" are always rows [0,1024): one uniform
SPMD program runs on all 8 cores.
"""

import numpy as np
from contextlib import ExitStack

import concourse.bass as bass
from concourse import bacc
import concourse.tile as tile
from concourse import mybir
from concourse.bass_utils import run_bass_kernel_spmd
from concourse.masks import make_identity

N, D, NOUT = 8192, 512, 512
M = 8                 # cores
R = N // M            # 1024 local rows per core
EPS = 1e-6
CLAMP = 0.3           # keeps the sqrt arg positive under fp16 gram noise
F32 = mybir.dt.float32
F16 = mybir.dt.float16

KT = D // 128         # 4 contraction sub-tiles
NJT = N // 128        # 64 j tiles
IB = 512              # i block (free dim of the gram matmuls)
NIB = R // IB         # 2
NSUB = IB // 128      # 4 sub-tiles of 128 rows per i block

CH = 512              # S^T DMA chunk width (columns); chunk c covers j_tiles 4c..4c+3
NCH = N // CH


def build_kernel(ctx, tc, out_d, xw_d, s_d, cj_d, ri_d, w2_d):
    nc = tc.nc

    singles = ctx.enter_context(tc.tile_pool(name="singles", bufs=1))
    g0_pool = ctx.enter_context(tc.tile_pool(name="g0", bufs=4))
    t_pool = ctx.enter_context(tc.tile_pool(name="t", bufs=3))
    osb_pool = ctx.enter_context(tc.tile_pool(name="osb", bufs=4))
    sm_pool = ctx.enter_context(tc.tile_pool(name="sm", bufs=4))
    macc_pool = ctx.enter_context(tc.tile_pool(name="macc", bufs=2))
    ps_tr = ctx.enter_context(tc.tile_pool(name="ps_tr", bufs=1, space="PSUM"))
    ps_g = ctx.enter_context(tc.tile_pool(name="ps_g", bufs=3, space="PSUM"))
    ps_y = ctx.enter_context(tc.tile_pool(name="ps_y", bufs=1, space="PSUM"))

    # --- persistent SBUF tensors ---
    st = singles.tile([128, KT * N], F16)             # S^T: [k*N + j] layout
    xw_sb = singles.tile([128, NJT * NOUT], F16)      # all xw tiles, resident
    cj_sb = singles.tile([128, NJT], F32)             # cj[t*128+p] at [p, t]
    ri_sb = singles.tile([128, R], F32)               # -ri/2, replicated rows
    w2_sb = singles.tile([128, NOUT], F32)            # w2 replicated rows
    ident = singles.tile([128, 128], F16)

    make_identity(nc, ident[:])

    # HAM warmup: dummy matmuls during the initial DMA window so the PE
    # clock-gate (4/8 cold -> 8/8 warm after ~3.4us of activity) is already
    # released when the first real gram matmul issues.
    dummy = singles.tile([128, 512], F16)
    nc.vector.memset(dummy[:], 0.015625)
    # touch Sqrt once so the ACT table load (~1.3us) happens during the
    # preamble instead of blocking the first real G0 tile
    actwarm = singles.tile([1, 8], F32)
    nc.scalar.activation(
        out=actwarm[:], in_=dummy[0:1, 0:8],
        func=mybir.ActivationFunctionType.Sqrt, scale=1.0,
    )
    for r in range(4):
        psw = ps_g.tile([128, 512], F32, tag="g", name=f"warm{r}")
        nc.tensor.matmul(psw[:], dummy[:, 0:128], dummy[:], start=True, stop=True)

    def load_st_chunk(c):
        # issue from the (otherwise idle) GPSIMD queue so descriptor issue
        # (~0.6us per DMA) does not serialize behind ri/cj/xw on Sync
        for k in range(KT):
            nc.gpsimd.dma_start(
                out=st[:, k * N + c * CH: k * N + (c + 1) * CH],
                in_=s_d[bass.ts(k, 128), c * CH:(c + 1) * CH],
            )

    nc.sync.dma_start(out=ri_sb[:], in_=ri_d)
    load_st_chunk(0)
    nc.sync.dma_start(out=cj_sb[:], in_=cj_d)

    # --- main: per i-block: gram strip -> G0 -> Y accum -> normalize ---
    for ib in range(NIB):
        icol0 = ib * IB  # local column offset into S^T / ri
        psy = [ps_y.tile([128, NOUT], F32, tag=f"y{s}", name=f"psy{s}")
               for s in range(NSUB)]
        macc = macc_pool.tile([128, IB], F16, tag="macc")
        pipe = []

        for jt in range(NJT):
            if ib == 0:
                nc.sync.dma_start(
                    out=xw_sb[:, jt * NOUT:(jt + 1) * NOUT],
                    in_=xw_d[bass.ts(jt, 128), :],
                )
                if jt == 0:
                    load_st_chunk(1)
                else:
                    # one k-slice per j-tile; chunk c complete at jt=4c-4,
                    # consumed from jt=4c (one-chunk slack, smooth DMA rate)
                    q = jt - 1
                    c = 2 + q // 4
                    if c < NCH:
                        k = q % 4
                        nc.gpsimd.dma_start(
                            out=st[:, k * N + c * CH: k * N + (c + 1) * CH],
                            in_=s_d[bass.ts(k, 128), c * CH:(c + 1) * CH],
                        )
                if jt == 32:
                    nc.sync.dma_start(out=w2_sb[:], in_=w2_d)

            psg = ps_g.tile([128, IB], F32, tag="g")
            for k in range(KT):
                nc.tensor.matmul(
                    psg[:],
                    st[:, k * N + jt * 128: k * N + jt * 128 + 128],
                    st[:, k * N + icol0: k * N + icol0 + IB],
                    start=(k == 0),
                    stop=(k == KT - 1),
                )

            # t = psg + (-ri/2)  (free-dim-varying term, DVE add)
            t = t_pool.tile([128, IB], F32, tag="t")
            nc.vector.tensor_add(t[:], psg[:], ri_sb[:, icol0:icol0 + IB])

            # G0^T tile = sqrt(-2*t + cj[j])   (cj includes +CLAMP)
            g0 = g0_pool.tile([128, IB], F16, tag="g0")
            nc.scalar.activation(
                out=g0[:], in_=t[:],
                func=mybir.ActivationFunctionType.Sqrt,
                bias=cj_sb[:, jt:jt + 1], scale=-2.0,
            )

            if jt == 0:
                nc.vector.tensor_copy(out=macc[:], in_=g0[:])
            else:
                nc.vector.tensor_max(macc[:], macc[:], g0[:])

            # software pipeline: issue Y matmuls three steps behind the gram so
            # the PE has two full j-tiles of slack over the DVE/ACT latency
            if jt > 2:
                pg0, pjt = pipe.pop(0)
                for s in range(NSUB):
                    nc.tensor.matmul(
                        psy[s][:], pg0[:, bass.ts(s, 128)],
                        xw_sb[:, pjt * NOUT:(pjt + 1) * NOUT],
                        start=(jt == 3), stop=False,
                    )
            pipe.append((g0, jt))

        # tail, s-major: finish psy[s] completely (epilogue Y MMs + macc
        # transpose on PE), then its rowmax/scale/store chain runs on DVE
        # while PE works on s+1. The last-s exposed tail is one chain only.
        pst = ps_tr.tile([128, IB], F16, tag="tr")
        for s in range(NSUB):
            for tail_i, (pg0, pjt) in enumerate(pipe):
                nc.tensor.matmul(
                    psy[s][:], pg0[:, bass.ts(s, 128)],
                    xw_sb[:, pjt * NOUT:(pjt + 1) * NOUT],
                    start=False, stop=(tail_i == len(pipe) - 1),
                )
            nc.tensor.transpose(
                pst[:, bass.ts(s, 128)], macc[:, bass.ts(s, 128)], ident[:])
            rm = sm_pool.tile([128, 1], F32, tag="rm")
            nc.vector.tensor_reduce(
                out=rm[:], in_=pst[:, bass.ts(s, 128)], axis=mybir.AxisListType.X,
                op=mybir.AluOpType.max,
            )
            nrm = sm_pool.tile([128, 1], F32, tag="nrm")
            nc.vector.tensor_scalar_mul(nrm[:], rm[:], -1.0)
            ninv = sm_pool.tile([128, 1], F32, tag="ninv")
            nc.vector.reciprocal(ninv[:], nrm[:])  # -1/rowmax
            osb = osb_pool.tile([128, NOUT], F32, tag="osb")
            nc.vector.scalar_tensor_tensor(
                out=osb[:], in0=psy[s][:], scalar=ninv[:], in1=w2_sb[:],
                op0=mybir.AluOpType.mult, op1=mybir.AluOpType.add,
            )
            nc.sync.dma_start(out=out_d[bass.ts(ib * NSUB + s, 128), :], in_=osb[:])


_NC_CACHE = {}


def _build_nc():
    if "nc" in _NC_CACHE:
        return _NC_CACHE["nc"]
    nc = bacc.Bacc("TRN2", target_bir_lowering=False, debug=False, num_devices=M)
    xw_d = nc.dram_tensor("xw", [N, NOUT], F16, kind="ExternalInput").ap()
    s_d = nc.dram_tensor("simT", [D, N], F16, kind="ExternalInput").ap()
    cj_d = nc.dram_tensor("cj", [128, NJT], F32, kind="ExternalInput").ap()
    ri_d = nc.dram_tensor("rirep", [128, R], F32, kind="ExternalInput").ap()
    w2_d = nc.dram_tensor("w2rep", [128, NOUT], F32, kind="ExternalInput").ap()
    out_d = nc.dram_tensor("out", [R, NOUT], F32, kind="ExternalOutput").ap()
    with tile.TileContext(nc) as tc, ExitStack() as ctx:
        build_kernel(ctx, tc, out_d, xw_d, s_d, cj_d, ri_d, w2_d)
    nc.compile()
    _NC_CACHE["nc"] = nc
    return nc


def make_in_maps(x, sim_feat, weight):
    x = np.ascontiguousarray(x, dtype=np.float32)
    sim = np.ascontiguousarray(sim_feat, dtype=np.float32)
    w = np.ascontiguousarray(weight, dtype=np.float32)

    sim64 = sim.astype(np.float64)
    sq = (sim64 * sim64).sum(1)
    ss = sim64.sum(1)
    cj_full = (sq - 2.0 * EPS * ss + CLAMP).astype(np.float32)         # [N]
    ri_full = sq + 2.0 * EPS * ss + D * EPS * EPS                      # [N] f64
    colsum = x.astype(np.float64).sum(0)
    w2 = (colsum @ w.astype(np.float64)).astype(np.float32)
    xw = (x @ w).astype(np.float16)                                    # [N, NOUT]
    sim16 = sim.astype(np.float16)
    w2_rep = np.ascontiguousarray(np.broadcast_to(w2, (128, NOUT)))

    in_maps = []
    for c in range(M):
        shift = c * R
        sim_c = np.ascontiguousarray(np.roll(sim16, -shift, axis=0).T)
        xw_c = np.roll(xw, -shift, axis=0)
        cj_c = np.ascontiguousarray(
            np.roll(cj_full, -shift).reshape(NJT, 128).T
        )                                                               # [128, NJT]
        ri_c = np.ascontiguousarray(np.broadcast_to(
            (-(ri_full[shift:shift + R]) / 2.0).astype(np.float32), (128, R)
        ))
        in_maps.append(
            {"xw": xw_c, "simT": sim_c, "cj": cj_c, "rirep": ri_c,
             "w2rep": w2_rep}
        )
    return in_maps


def _ensure_ntff_hook():
    """bass_utils' trace path hard-imports antenv.axon_hooks, which some agent
    images lack. Provide it (with the real ctypes NTFF hook when the axon .so
    is present) so a BASS_TRACE=1 environment doesn't crash the kernel."""
    import sys, types
    try:
        import antenv.axon_hooks  # noqa: F401
        return
    except ImportError:
        pass
    try:
        import antenv
    except ImportError:
        return
    mod = types.ModuleType("antenv.axon_hooks")
    _state = {"hook": None}
    mod.set_axon_ntff_profile_hook = lambda h: _state.__setitem__("hook", h)
    mod.get_axon_ntff_profile_hook = lambda: _state["hook"]
    sys.modules["antenv.axon_hooks"] = mod
    antenv.axon_hooks = mod
    try:
        import os
        from trn_agent_boot.trn_boot import _ntff_profile_via_ctypes
        so = "/opt/axon/libaxon_pjrt.so"
        if os.path.exists(so):
            mod.set_axon_ntff_profile_hook(_ntff_profile_via_ctypes(so))
    except Exception:
        pass


def kernel(x, sim_feat, weight, _trace=False, **kw):
    _ensure_ntff_hook()
    nc = _build_nc()
    in_maps = make_in_maps(x, sim_feat, weight)
    res = run_bass_kernel_spmd(nc, in_maps, list(range(M)), trace=_trace, **kw)
    out = np.concatenate([res.results[c]["out"] for c in range(M)], axis=0)
    if _trace:
        return out, res
    return out
